# revision 8
# baseline (speedup 1.0000x reference)
"""Bass/Tile kernel builder for sharded MultiHeadAttention on TRN2.

Sharding: 8 cores = 2 batches x 4 head-groups (4 heads each, e-slice of 256).
Each core computes a partial output outT [1024, 2048] (bf16, transposed);
host sums the 4 head-group partials per batch and transposes back.

Schedule highlights:
  - weights/biases/ones loaded ONCE per NEFF (outside the rep loop)
  - heads processed in PAIRS: head h%2==0 lives on SBUF partitions 0-63,
    h%2==1 on 64-127, so the pair's K=64 QK matmuls land on PE row-tiles
    (0,0)/(64,0) and stream CONCURRENTLY (2x QK throughput)
  - attention tiled as (i-chunk 512) x (j-block-pair 256): each S tile
    [128, 1024] holds S^T for two j-blocks at one i-chunk, so psS can be
    triple-buffered (6 banks) and exp latency hides behind the PE stream,
    while each head's O accumulator is a single PSUM bank (psO 2 banks)
  - softmax exp split across engines: ~5/8 of tiles on ACT (exact exp),
    ~3/8 on DVE via a Schraudolph bit-trick (int16(S*128*log2e + 16249.3)
    viewed as bf16 == 2^(S*log2e) to ~1.8% rms, zero-mean after softmax)
  - normalize: reciprocal of the denominator row + gpsimd broadcast, then
    ONE fused scalar_tensor_tensor drains O from PSUM into A = O * (1/d)
  - K/Q bias-adds and denominator copies alternate ACT/DVE to balance
  - V-proj runs through the psK pool during the projection phase
  - x tensors for rep r+1 prefetched mid-rep
"""
from contextlib import ExitStack

import concourse.bass as bass
import concourse.tile as tile
from concourse import bacc, mybir

F32 = mybir.dt.float32
BF16 = mybir.dt.bfloat16
I16 = mybir.dt.int16

T = 2048          # sequence length
D = 1024          # d_model
E = 256           # per-core projection width (4 heads x 64)
HPC = 4           # heads per core
DH = 64           # head dim
KB = D // 128     # contraction blocks for projections
TB = T // 128     # t-blocks / j-blocks
ICN = 512         # attention i-chunk
NI = T // ICN     # 4 i-chunks
JP = TB // 2      # 8 j-block pairs
PRE_BUFS = 24     # P-tile slots
PRE_ICN = 2       # i-chunks of pair 0 prefetched during the proj phase

# Schraudolph-exp constants for bf16-viewed int16:
#   i16 = S*128*log2(e) + (127*128 - 6.7)  -> bitcast bf16 ~= exp(S)
SCH_A = 184.6649652337873
SCH_B = 16249.3
# j-block-pairs whose exp goes to DVE (even head / odd head of each pair)
DVE_E = frozenset((1, 3, 5))
DVE_O = frozenset((2, 4, 6))


class _Bacc(bacc.Bacc):
    # Keep matmul waits on the MATMUL so LDWEIGHTS stays wait-free and can
    # pre-load during the previous matmul (excess waits become EVSEMs).
    def move_matmul_waits_to_ldweights(self):
        pass


def build_nc(reps=1):
    nc = _Bacc("TRN2", target_bir_lowering=False, debug=False,
               enable_asserts=False, num_devices=8)
    din = {}
    for name in ("xqT", "xkT", "xvT"):
        din[name] = nc.dram_tensor(name, [D, T], BF16, kind="ExternalInput").ap()
    for name in ("wqT", "wkT", "wvT"):
        din[name] = nc.dram_tensor(name, [D, E], BF16, kind="ExternalInput").ap()
    din["woT"] = nc.dram_tensor("woT", [E, D], BF16, kind="ExternalInput").ap()
    din["bq"] = nc.dram_tensor("bq", [E], F32, kind="ExternalInput").ap()
    din["bk"] = nc.dram_tensor("bk", [E], F32, kind="ExternalInput").ap()
    din["bv"] = nc.dram_tensor("bv", [E], F32, kind="ExternalInput").ap()
    din["bo"] = nc.dram_tensor("bo", [D], F32, kind="ExternalInput").ap()
    outT = nc.dram_tensor("outT", [D, T], BF16, kind="ExternalOutput").ap()

    with tile.TileContext(nc) as tc:
        _build(tc, nc, din, outT, reps)
    nc.compile()
    return nc


def _build(tc, nc, din, outT, reps):
    with ExitStack() as ctx:
        per = ctx.enter_context(tc.tile_pool(name="per", bufs=1))

        # ---- persistent SBUF tensors: weights/biases/ones, loaded once ----
        wq = per.tile([128, KB, E], BF16, tag="wq")
        wk = per.tile([128, KB, E], BF16, tag="wk")
        wv = per.tile([128, KB, E], BF16, tag="wv")
        wo = per.tile([128, 2, D], BF16, tag="wo")
        bq = per.tile([128, 2], F32, tag="bq")
        bk = per.tile([128, 2], F32, tag="bk")
        bvb = per.tile([128, E], F32, tag="bvb")
        bo = per.tile([128, KB], F32, tag="bo")
        vv = per.tile([128, TB, HPC * (DH + 1)], BF16, tag="vv")

        nc.sync.dma_start(bq[:], din["bq"].rearrange("(a p) -> p a", p=128))
        nc.sync.dma_start(bk[:], din["bk"].rearrange("(a p) -> p a", p=128))
        nc.sync.dma_start(bo[:], din["bo"].rearrange("(a p) -> p a", p=128))
        nc.sync.dma_start(bvb[:], din["bv"].partition_broadcast(128))
        # ones columns for the denominator trick (V columns rewritten per rep)
        nc.vector.memset(vv[:], 1.0)

        # PE clock warmup: dummy accumulating matmuls on the ones tile while
        # the first x chunks are still in flight (HAM ramps on busy time)
        with tc.tile_pool(name="psW", bufs=1, space="PSUM") as psW:
            wrm = psW.tile([128, 512], F32, tag="W")
            for i in range(40):
                nc.tensor.matmul(wrm[:, 0:256], vv[:, 0, 0:128],
                                 vv[:, 1, 0:256],
                                 start=(i == 0), stop=(i == 39))

        # double-buffered per-rep tensors
        sbKQ = ctx.enter_context(tc.tile_pool(name="sbKQ", bufs=2))
        sbA = ctx.enter_context(tc.tile_pool(name="sbA", bufs=2))
        sbX = ctx.enter_context(tc.tile_pool(name="sbX", bufs=2))

        def alloc_x(r):
            """Allocate + DMA the rep-r input tiles."""
            xk = sbX.tile([128, KB, T], BF16, tag="x", name=f"xk{r}")
            xq = sbX.tile([128, KB, T], BF16, tag="x", name=f"xq{r}")
            xv = sbX.tile([128, KB, T], BF16, tag="x", name=f"xv{r}")
            for xname, xdst in (("xkT", xk), ("xqT", xq), ("xvT", xv)):
                src = din[xname].rearrange("(kb p) t -> p kb t", p=128)
                for kb in range(KB):
                    nc.sync.dma_start(xdst[:, kb, :], src[:, kb, :])
            return xk, xq, xv

        # rep-0 inputs, ordered so the K-path lands first
        xk0 = sbX.tile([128, KB, T], BF16, tag="x", name="xk0")
        xq0 = sbX.tile([128, KB, T], BF16, tag="x", name="xq0")
        xv0 = sbX.tile([128, KB, T], BF16, tag="x", name="xv0")
        wsrc = {n: din[n].rearrange("(kb p) t -> p kb t", p=128)
                for n in ("wkT", "wqT", "wvT", "woT")}
        xsrc = {n: din[n].rearrange("(kb p) t -> p kb t", p=128)
                for n in ("xkT", "xqT", "xvT")}
        for kb in range(KB):
            nc.sync.dma_start(wk[:, kb, :], wsrc["wkT"][:, kb, :])
        for kb in range(KB):
            nc.sync.dma_start(xk0[:, kb, :], xsrc["xkT"][:, kb, :])
        nc.sync.dma_start(wq[:], wsrc["wqT"])
        for kb in range(KB):
            nc.sync.dma_start(xq0[:, kb, :], xsrc["xqT"][:, kb, :])
        nc.sync.dma_start(wv[:], wsrc["wvT"])
        for kb in range(KB):
            nc.sync.dma_start(xv0[:, kb, :], xsrc["xvT"][:, kb, :])
        nc.sync.dma_start(wo[:], wsrc["woT"])

        xt = (xk0, xq0, xv0)
        for r in range(reps):
            nxt = [None]

            def prefetch(r=r):
                if r + 1 < reps:
                    nxt[0] = alloc_x(r + 1)

            _rep(tc, nc, din, outT, r,
                 wq, wk, wv, wo, bq, bk, bvb, bo, vv, sbKQ, sbA,
                 xt, prefetch)
            xt = nxt[0]


def _rep(tc, nc, din, outT, r,
         wq, wk, wv, wo, bq, bk, bvb, bo, vv, sbKQ, sbA, xt, prefetch):
    Exp = mybir.ActivationFunctionType.Exp
    Ident = mybir.ActivationFunctionType.Identity
    Mult = mybir.AluOpType.mult
    xk, xq, xv = xt

    kt = sbKQ.tile([128, 2, T], BF16, tag="kt", name=f"kt{r}")
    qt = sbKQ.tile([128, 2, T], BF16, tag="qt", name=f"qt{r}")
    aa = sbA.tile([128, 2, T], BF16, tag="aa", name=f"aa{r}")

    with ExitStack() as ctx:
        actx = ctx.enter_context(ExitStack())
        psS = actx.enter_context(
            tc.tile_pool(name=f"psS{r}", bufs=3, space="PSUM"))
        sbP = actx.enter_context(tc.tile_pool(name=f"sbP{r}", bufs=PRE_BUFS))
        sbN = actx.enter_context(tc.tile_pool(name=f"sbN{r}", bufs=2))

        def qk_exp(h, icn, p, dve):
            """S^T for j-blocks {2p, 2p+1} x i-chunk icn, then exp."""
            et, eo = h // 2, (h % 2) * 64
            st = psS.tile([128, 2, ICN], F32, tag="S",
                          name=f"st{h}_{icn}_{p}_{r}")
            for half in range(2):
                jb = 2 * p + half
                nc.tensor.matmul(
                    st[:, half, :],
                    kt[eo:eo + DH, et, jb * 128:(jb + 1) * 128],
                    qt[eo:eo + DH, et, icn * ICN:(icn + 1) * ICN],
                    start=True, stop=True)
            if dve:
                pt = sbP.tile([128, 2, ICN], I16, tag="P",
                              name=f"pt{h}_{icn}_{p}_{r}")
                nc.vector.tensor_scalar(
                    pt[:], st[:], SCH_A, SCH_B, Mult, mybir.AluOpType.add)
                return pt[:].bitcast(BF16)
            pt = sbP.tile([128, 2, ICN], BF16, tag="P",
                          name=f"pt{h}_{icn}_{p}_{r}")
            nc.scalar.activation(pt[:], st[:], Exp)
            return pt[:]

        with tc.tile_pool(name=f"psK{r}", bufs=2, space="PSUM") as psK:

            def proj_pass(pname, w_t, b_t, src, dst, et):
                """One et-pass of a K/Q projection: 2x2 psum groups x 8 kb,
                PSUM->SBUF bias-moves alternating DVE/ACT."""
                for half in range(2):
                    pss = [psK.tile([128, 512], F32, tag="pp",
                                    name=f"pp_{pname}{et}_{half}_{g}_{r}")
                           for g in range(2)]
                    for kb in range(KB):
                        for g in range(2):
                            nch = half * 2 + g
                            nc.tensor.matmul(
                                pss[g][:],
                                w_t[:, kb, et * 128:(et + 1) * 128],
                                src[:, kb, nch * 512:(nch + 1) * 512],
                                start=(kb == 0), stop=(kb == KB - 1))
                    for g in range(2):
                        nch = half * 2 + g
                        dstc = dst[:, et, nch * 512:(nch + 1) * 512]
                        if g == 0:
                            nc.vector.tensor_scalar_add(
                                dstc, pss[g][:], b_t[:, et:et + 1])
                        else:
                            nc.scalar.activation(dstc, pss[g][:], Ident,
                                                 bias=b_t[:, et:et + 1])

            # et0 passes (heads 0,1), then the prefetched pair-0 QK+exp
            # chunks (feeds ACT/DVE during the remaining proj work), then et1
            proj_pass("k", wk, bk, xk, kt, 0)
            proj_pass("q", wq, bq, xq, qt, 0)
            early = []
            for icn in range(PRE_ICN):
                for p in range(JP):
                    pe_ = qk_exp(0, icn, p, dve=(p in DVE_E))
                    po_ = qk_exp(1, icn, p, dve=(p in DVE_O))
                    early.append((pe_, po_))
            proj_pass("k", wk, bk, xk, kt, 1)
            proj_pass("q", wq, bq, xq, qt, 1)

            # V projection through the psK slots: 8 waves of 2 t-blocks
            bvb4 = bvb[:, :].rearrange("p (h d) -> p h d", h=HPC)
            for wave in range(8):
                tbs = [wave * 2, wave * 2 + 1]
                psv = [psK.tile([128, 512], F32, tag="pp", name=f"vp_{tb}_{r}")
                       for tb in tbs]
                for kb in range(KB):
                    for i, tb in enumerate(tbs):
                        nc.tensor.matmul(
                            psv[i][:, 0:E],
                            xv[:, kb, tb * 128:(tb + 1) * 128],
                            wv[:, kb, :],
                            start=(kb == 0), stop=(kb == KB - 1))
                for i, tb in enumerate(tbs):
                    dstv = vv[:, tb, :].rearrange(
                        "p (h x) -> p h x", h=HPC)[:, :, 0:DH]
                    srcv = psv[i][:, 0:E].rearrange("p (h d) -> p h d", h=HPC)
                    nc.vector.tensor_add(dstv, srcv, bvb4)

        # ---- attention ----
        with tc.tile_pool(name=f"psO{r}", bufs=2, space="PSUM") as psO:

            def pv(h, oc, p, pt):
                for half in range(2):
                    jb = 2 * p + half
                    nc.tensor.matmul(
                        oc[0:DH + 1, :],
                        vv[:, jb, h * (DH + 1):(h + 1) * (DH + 1)],
                        pt[:, half, :],
                        start=(p == 0 and half == 0),
                        stop=(p == JP - 1 and half == 1))

            def normalize(h, icn, oc, use_act):
                et, eo = h // 2, (h % 2) * 64
                dcp = sbN.tile([1, ICN], F32, tag="dcp")
                if use_act:
                    nc.scalar.copy(dcp[:], oc[DH:DH + 1, :])
                else:
                    nc.vector.tensor_copy(dcp[:], oc[DH:DH + 1, :])
                rr = sbN.tile([1, ICN], F32, tag="rr")
                nc.vector.reciprocal_approx_fast(rr[:], dcp[:])
                rb = sbN.tile([DH, ICN], F32, tag="rb")
                nc.gpsimd.partition_broadcast(rb[:], rr[:])
                # A = (O * 1.0) * (1/d): one fused op drains PSUM
                nc.vector.scalar_tensor_tensor(
                    aa[eo:eo + DH, et, icn * ICN:(icn + 1) * ICN],
                    oc[0:DH, :], 1.0, rb[:], Mult, Mult)

            # prefetch next rep's inputs: emitted here so the scheduler
            # gives the dispatches mid-rep priority
            prefetch()

            for pr in range(2):
                he, ho = 2 * pr, 2 * pr + 1
                for icn in range(NI):
                    oc_e = psO.tile([128, ICN], F32, tag="O",
                                    name=f"oc{he}_{icn}_{r}")
                    oc_o = psO.tile([128, ICN], F32, tag="O",
                                    name=f"oc{ho}_{icn}_{r}")
                    for p in range(JP):
                        if pr == 0 and icn < PRE_ICN:
                            pe_, po_ = early[icn * JP + p]
                        else:
                            pe_ = qk_exp(he, icn, p, dve=(p in DVE_E))
                            po_ = qk_exp(ho, icn, p, dve=(p in DVE_O))
                        pv(he, oc_e, p, pe_)
                        pv(ho, oc_o, p, po_)
                    normalize(he, icn, oc_e, use_act=(icn % 2 == 0))
                    normalize(ho, icn, oc_o, use_act=(icn % 2 == 1))

        actx.close()  # free psS/sbP/sbN before the oproj staging pools

        # ---- output projection ----
        with tc.tile_pool(name=f"psC{r}", bufs=4, space="PSUM") as psC, \
             tc.tile_pool(name=f"sbO{r}", bufs=4) as sbO:
            for ft in range(KB):  # 8 f-blocks of 128
                stg = sbO.tile([128, T], BF16, tag="stg")
                for nch in range(4):  # t chunks of 512
                    ps = psC.tile([128, 512], F32, tag="op")
                    for kb in range(2):
                        nc.tensor.matmul(
                            ps[:],
                            wo[:, kb, ft * 128:(ft + 1) * 128],
                            aa[:, kb, nch * 512:(nch + 1) * 512],
                            start=(kb == 0), stop=(kb == 1))
                    # alternate PSUM->SBUF bias-add between DVE and ACT
                    dst = stg[:, nch * 512:(nch + 1) * 512]
                    if nch % 2 == 0:
                        nc.vector.tensor_scalar_add(dst, ps[:], bo[:, ft:ft + 1])
                    else:
                        nc.scalar.activation(dst, ps[:], Ident,
                                             bias=bo[:, ft:ft + 1])
                nc.sync.dma_start(
                    outT.rearrange("(ft p) t -> p ft t", p=128)[:, ft, :],
                    stg[:])


# ======================== host-side wrapper ========================
import numpy as np
import ml_dtypes

NP_BF16 = ml_dtypes.bfloat16
B = 2
NCORES = 8
GPB = 4
_CACHE = {}


def _core_inputs(c, q, k, v, Wq, bq, Wk, bk, Wv, bv, Wo, bo):
    b, g = divmod(c, GPB)
    es = slice(g * E, g * E + E)
    return {
        "xqT": np.ascontiguousarray(q[b].T).astype(NP_BF16),
        "xkT": np.ascontiguousarray(k[b].T).astype(NP_BF16),
        "xvT": np.ascontiguousarray(v[b].T).astype(NP_BF16),
        "wqT": np.ascontiguousarray((Wq[es, :] / 8.0).T).astype(NP_BF16),
        "wkT": np.ascontiguousarray(Wk[es, :].T).astype(NP_BF16),
        "wvT": np.ascontiguousarray(Wv[es, :].T).astype(NP_BF16),
        "woT": np.ascontiguousarray(Wo[:, es].T).astype(NP_BF16),
        "bq": (np.asarray(bq)[es] / 8.0).astype(np.float32),
        "bk": np.asarray(bk)[es].astype(np.float32),
        "bv": np.asarray(bv)[es].astype(np.float32),
        "bo": (np.asarray(bo) if g == 0 else np.zeros_like(bo)).astype(np.float32),
    }


def kernel(q, k, v, Wq, bq, Wk, bk, Wv, bv, Wo, bo):
    """Full-input MultiHeadAttention on 8 NeuronCores; returns [2,2048,1024] f32."""
    from concourse.bass_utils import run_bass_kernel_spmd

    if "nc" not in _CACHE:
        _CACHE["nc"] = build_nc()
    nc = _CACHE["nc"]

    args = dict(q=np.asarray(q, np.float32), k=np.asarray(k, np.float32),
                v=np.asarray(v, np.float32), Wq=np.asarray(Wq, np.float32),
                bq=np.asarray(bq, np.float32), Wk=np.asarray(Wk, np.float32),
                bk=np.asarray(bk, np.float32), Wv=np.asarray(Wv, np.float32),
                bv=np.asarray(bv, np.float32), Wo=np.asarray(Wo, np.float32),
                bo=np.asarray(bo, np.float32))
    in_maps = [_core_inputs(c, **args) for c in range(NCORES)]
    res = run_bass_kernel_spmd(nc, in_maps, core_ids=list(range(NCORES)))
    out = np.zeros((B, T, D), np.float32)
    for c, r in enumerate(res.results):
        out[c // GPB] += r["outT"].T.astype(np.float32)
    return out
